# revision 2
# baseline (speedup 1.0000x reference)
"""ArcFace head (B=1024, D=512, C=100000) on 8 TRN2 NeuronCores.

Sharding: tensor-parallel along the num_classes axis (partial-FC ArcFace).
Each core holds a [D, C/8] slice of the (pre-normalized, pre-transposed)
weight and computes its [B, C/8] slice of S * cosine via a bf16 TensorE
matmul with fp32 PSUM accumulation. Embeddings (normalized, scaled by S,
transposed to [D, B]) are broadcast to all cores. The per-row additive
angular margin touches exactly B elements of the [B, C] output, so it is
applied on the host after the gather (exact trig identity:
cos(theta + m) = c*cos(m) - sqrt(1-c^2)*sin(m)).
"""

import numpy as np
import ml_dtypes

import concourse.bass as bass
import concourse.mybir as mybir
from concourse import bacc
from concourse.tile import TileContext
from concourse.bass import ts
from concourse.bass_utils import run_bass_kernel_spmd

# Problem constants (hardcoded per spec)
B, D, C = 1024, 512, 100000
NCORES = 8
CS = C // NCORES          # 12500 classes per core
S, MARGIN, EPS = 30.0, 0.5, 1e-7

P = 128                   # partitions
KS = D // P               # 4 k-subtiles
MS = B // P               # 8 m-subtiles
NT = 512                  # n tile (one PSUM bank of fp32)
N_TILES = (CS + NT - 1) // NT  # 25 (last tile 212 wide)

BF16 = mybir.dt.bfloat16
_bf16_np = ml_dtypes.bfloat16


def build_nc():
    nc = bacc.Bacc(None, target_bir_lowering=False)
    embT = nc.dram_tensor("embT", [D, B], BF16, kind="ExternalInput")
    wT = nc.dram_tensor("wT", [D, CS], BF16, kind="ExternalInput")
    out = nc.dram_tensor("out", [B, CS], BF16, kind="ExternalOutput")

    with TileContext(nc) as tc:
        with (
            tc.tile_pool(name="emb", bufs=1) as epool,
            tc.tile_pool(name="w", bufs=3) as wpool,
            tc.tile_pool(name="o", bufs=3) as opool,
            tc.tile_pool(name="ps", bufs=8, space="PSUM") as pspool,
        ):
            emb_sb = epool.tile([P, KS, B], BF16)
            nc.sync.dma_start(
                out=emb_sb[:], in_=embT[:].rearrange("(ko p) b -> p ko b", p=P)
            )
            wT_r = wT[:].rearrange("(ko p) c -> p ko c", p=P)
            out_r = out[:].rearrange("(mo p) c -> p mo c", p=P)
            for n in range(N_TILES):
                n0 = n * NT
                nw = min(NT, CS - n0)
                w_sb = wpool.tile([P, KS, NT], BF16, tag="w")
                nc.sync.dma_start(out=w_sb[:, :, :nw], in_=wT_r[:, :, n0 : n0 + nw])
                o_sb = opool.tile([P, MS, NT], BF16, tag="o")
                for m in range(MS):
                    ps = pspool.tile([P, NT], mybir.dt.float32, tag="ps")
                    for k in range(KS):
                        nc.tensor.matmul(
                            ps[:, :nw],
                            lhsT=emb_sb[:, k, ts(m, P)],
                            rhs=w_sb[:, k, :nw],
                            start=(k == 0),
                            stop=(k == KS - 1),
                        )
                    # split PSUM->SBUF copies between ACT and DVE
                    if m % 2 == 0:
                        nc.scalar.copy(out=o_sb[:, m, :nw], in_=ps[:, :nw])
                    else:
                        nc.vector.tensor_copy(out=o_sb[:, m, :nw], in_=ps[:, :nw])
                nc.sync.dma_start(
                    out=out_r[:, :, n0 : n0 + nw], in_=o_sb[:, :, :nw]
                )
    nc.finalize()
    return nc


_NC_CACHE = []


def _get_nc():
    if not _NC_CACHE:
        _NC_CACHE.append(build_nc())
    return _NC_CACHE[0]


def _prep_in_maps(embeddings, weight):
    # normalize on host (fp32), fold the ArcFace scale S into the embeddings
    en = embeddings / np.maximum(
        np.linalg.norm(embeddings, axis=1, keepdims=True), 1e-12
    )
    wn = weight / np.maximum(np.linalg.norm(weight, axis=1, keepdims=True), 1e-12)
    embT = np.ascontiguousarray((S * en).T).astype(_bf16_np)  # [D, B]
    wTn = wn.T  # [D, C] view
    in_maps = []
    for i in range(NCORES):
        shard = np.ascontiguousarray(wTn[:, i * CS : (i + 1) * CS]).astype(_bf16_np)
        in_maps.append({"embT": embT, "wT": shard})
    return in_maps


def run_device(embeddings, weight, **spmd_kwargs):
    """Runs the device part; returns (full S*cosine [B, C] fp32, raw results)."""
    nc = _get_nc()
    in_maps = _prep_in_maps(embeddings, weight)
    res = run_bass_kernel_spmd(nc, in_maps, core_ids=list(range(NCORES)), **spmd_kwargs)
    out = np.concatenate(
        [np.asarray(res.results[i]["out"]).astype(np.float32) for i in range(NCORES)],
        axis=1,
    )
    return out, res


def apply_margin(out, labels):
    rows = np.arange(B)
    lab = np.asarray(labels).astype(np.int64)
    c = np.clip(out[rows, lab] / S, -1.0 + EPS, 1.0 - EPS)
    out[rows, lab] = S * (c * np.cos(MARGIN) - np.sqrt(1.0 - c * c) * np.sin(MARGIN))
    return out


def kernel(embeddings, weight, labels):
    embeddings = np.asarray(embeddings, dtype=np.float32)
    weight = np.asarray(weight, dtype=np.float32)
    out, _ = run_device(embeddings, weight)
    return apply_margin(out, labels)
